# revision 57
# baseline (speedup 1.0000x reference)
"""TRN2 Bass kernel for nn_Attention (Luong 'general' global attention).

reference:
    h_t    = input @ affine_w.T + affine_b          [B,T,H]
    scores = h_t @ context.T                        [B,T,S]
    align  = softmax(scores, axis=S)
    c      = align @ context                        [B,T,H]
    out    = tanh(concat([c, input]) @ mlp_w.T + mlp_b)

B=16, T=1024, S=2048, H=1024. Data-parallel over batch: 2 batches/core
on 8 NeuronCores, no collectives. Compute dtype fp16 (TensorE full
rate, f32 PSUM accumulate); softmax row-stats in f32.

Per-core dataflow (per local batch):
  1. h_tT[o,t]  = affine_wT-matmuls(inputT) + bias       (psum->sbuf fp16)
  2. scores[t,s] = h_tT.T @ contextT   per t-tile of 128 (psum->sbuf f32)
  3. rowmax/exp/rowsum via DVE reduce + ACT exp(accum_out), recip via DVE
  4. alignT[s,t] via PE transpose;  cT'[h,t] = context.T-matmuls(alignT)
  5. out[t,o] = tanh(cT'-matmuls*recip + inputT/W2-matmuls + mlp_b)

The emission order software-pipelines the PE stream across t-tiles,
t-chunks AND batches: transposes of tile i run behind the score matmuls
of tile i+1; phase 4+5 of chunk c run behind the first score group of
chunk c+1; phase 1 of batch b+1 fills the tail of batch b.
"""
import sys

sys.path.insert(0, "/opt/trn_rl_repo")
import numpy as np  # noqa: E402
from concourse import bacc, bass, tile, masks  # noqa: E402
from concourse.bass_utils import run_bass_kernel_spmd  # noqa: E402

mybir = bass.mybir
F16 = mybir.dt.float16
F32 = mybir.dt.float32
AF = mybir.ActivationFunctionType

N_CORES = 8
B, T, S, H = 16, 1024, 2048, 1024
B_LOC = B // N_CORES          # 2 batches per core
KT = H // 128                 # 8 contraction tiles
TT = T // 128                 # 8 t-tiles per batch
TCH = T // 512                # 2 t-chunks per batch
OCH = H // 512                # 2 output chunks
SCH = S // 512                # 4 score chunks
SBLK = S // 128               # 16 s blocks

_nc_cache = None


def build():
    nc = bacc.Bacc("TRN2", target_bir_lowering=False, debug=False,
                   num_devices=N_CORES)
    inputT_d = nc.declare_dram_parameter("inputT", [B_LOC, H, T], F16, isOutput=False)
    contextT_d = nc.declare_dram_parameter("contextT", [B_LOC, H, S], F16, isOutput=False)
    context_d = nc.declare_dram_parameter("context", [B_LOC, S, H], F16, isOutput=False)
    awT_d = nc.declare_dram_parameter("affine_wT", [H, H], F16, isOutput=False)
    ab_d = nc.declare_dram_parameter("affine_b", [H, 1], F32, isOutput=False)
    w1T_d = nc.declare_dram_parameter("w1T", [H, H], F16, isOutput=False)
    w2T_d = nc.declare_dram_parameter("w2T", [H, H], F16, isOutput=False)
    mb_d = nc.declare_dram_parameter("mlp_b", [128, H], F16, isOutput=False)
    out_d = nc.declare_dram_parameter("out", [B_LOC * T, H], F32, isOutput=True)

    with tile.TileContext(nc) as tc:
        with tc.tile_pool(name="const", bufs=1) as cpool, \
             tc.tile_pool(name="big", bufs=1) as bpool, \
             tc.tile_pool(name="align", bufs=1) as apool, \
             tc.tile_pool(name="work", bufs=2) as wpool, \
             tc.tile_pool(name="out", bufs=2) as opool, \
             tc.tile_pool(name="small", bufs=4) as spool, \
             tc.tile_pool(name="ps", bufs=8, space="PSUM") as ps:

            # HWDGE DMAs are FIFO per issuing engine (two rings: sync, scalar)
            # -> few big DMAs, critical ones first, spread over both rings.
            def load_kxn(pool, dram, n, dt, tag, groups=8, eng=None):
                t_ = pool.tile([128, KT * n], dt, tag=tag)
                v = t_[:].rearrange("p (k n) -> p k n", k=KT)
                kg = KT // groups
                for g in range(groups):
                    (eng or nc.sync).dma_start(
                        out=v[:, g * kg:(g + 1) * kg, :],
                        in_=dram[:].rearrange("(k p) n -> p k n", p=128)
                        [:, g * kg:(g + 1) * kg, :])
                return v

            def load_inputT_half(b, hh):
                ih = bpool.tile([128, KT * 512], F16, tag=f"inputT{hh}")
                ihv = ih[:].rearrange("p (k t) -> p k t", k=KT)
                for k in range(KT):
                    nc.sync.dma_start(
                        out=ihv[:, k, :],
                        in_=inputT_d[b].rearrange("(k p) t -> p k t", p=128)
                        [:, k, hh * 512:(hh + 1) * 512])
                return ihv

            def load_contexts(b):
                contextT = bpool.tile([128, KT * S], F16, tag="contextT")
                cTv = contextT[:].rearrange("p (k s) -> p k s", k=KT)
                for k in range(KT):
                    nc.sync.dma_start(
                        out=cTv[:, k, :],
                        in_=contextT_d[b].rearrange("(k p) s -> p k s", p=128)[:, k, :])
                context = bpool.tile([128, SBLK * H], F16, tag="context")
                cv = context[:].rearrange("p (s h) -> p s h", s=SBLK)
                for sb in range(SBLK):
                    nc.sync.dma_start(
                        out=cv[:, sb, :],
                        in_=context_d[b].rearrange("(s p) h -> p s h", p=128)[:, sb, :])
                return cTv, cv

            def emit_phase1(inputT_h, htT_v, chunks=None):
                for tc2 in (range(TCH) if chunks is None else chunks):
                    for o in range(KT):
                        acc = ps.tile([128, 512], F32, tag="ps")
                        for k in range(KT):
                            nc.tensor.matmul(
                                acc[:, :],
                                awT[:, k, o * 128:(o + 1) * 128],
                                inputT_h[tc2][:, k, :],
                                start=(k == 0), stop=(k == KT - 1))
                        nc.vector.tensor_scalar_add(
                            htT_v[:, o, tc2 * 512:(tc2 + 1) * 512],
                            acc[:, :], ab[:, o:o + 1])

            # ---- HAM pre-warm: PE busy during initial loads so the
            # clock gate opens (K=8/8) before the first real matmul ----
            warm = cpool.tile([128, 512], F16, tag="warm")
            nc.vector.memset(warm[:, :], 0.5)
            wps = ps.tile([128, 512], F32, tag="ps")
            for i in range(12):
                nc.tensor.matmul(wps[:, :], warm[:, 0:128], warm[:, :],
                                 start=(i == 0), stop=(i == 11))

            # ---- batch-0 critical-path loads first ----
            # awT split across both HWDGE rings (scalar ring idle at t=0)
            state = {}
            awT_t = cpool.tile([128, KT * H], F16, tag="awT")
            awT = awT_t[:].rearrange("p (k n) -> p k n", k=KT)
            for g in range(2):
                nc.sync.dma_start(
                    out=awT[:, g * 4:(g + 1) * 4, :],
                    in_=awT_d[:].rearrange("(k p) n -> p k n", p=128)
                    [:, g * 4:(g + 1) * 4, :])
            ab = cpool.tile([128, KT], F32, tag="ab")
            nc.sync.dma_start(out=ab[:, :],
                              in_=ab_d[:].rearrange("(o p) one -> p (o one)", p=128))
            # order: inputT half 0 (phase 1 tc0), contextT (first scores),
            # inputT half 1, context, then phase-5 weights
            ih0 = bpool.tile([128, KT * 512], F16, tag="inputT0")
            ih0v = ih0[:].rearrange("p (k t) -> p k t", k=KT)
            for g in range(2):
                nc.scalar.dma_start(
                    out=ih0v[:, g * 4:(g + 1) * 4, :],
                    in_=inputT_d[0].rearrange("(k p) t -> p k t", p=128)
                    [:, g * 4:(g + 1) * 4, 0:512])
            contextT0 = bpool.tile([128, KT * S], F16, tag="contextT")
            cTv0 = contextT0[:].rearrange("p (k s) -> p k s", k=KT)
            for k in range(KT):
                nc.sync.dma_start(
                    out=cTv0[:, k, :],
                    in_=contextT_d[0].rearrange("(k p) s -> p k s", p=128)[:, k, :])
            ih1 = bpool.tile([128, KT * 512], F16, tag="inputT1")
            ih1v = ih1[:].rearrange("p (k t) -> p k t", k=KT)
            for k in range(KT):
                nc.scalar.dma_start(
                    out=ih1v[:, k, :],
                    in_=inputT_d[0].rearrange("(k p) t -> p k t", p=128)
                    [:, k, 512:1024])
            context0 = bpool.tile([128, SBLK * H], F16, tag="context")
            cv0 = context0[:].rearrange("p (s h) -> p s h", s=SBLK)
            for sb in range(SBLK):
                nc.sync.dma_start(
                    out=cv0[:, sb, :],
                    in_=context_d[0].rearrange("(s p) h -> p s h", p=128)[:, sb, :])
            state[0] = ([ih0v, ih1v], cTv0, cv0)
            # phase-5 constants after the phase-1/2-critical DMAs
            w1T = load_kxn(cpool, w1T_d, H, F16, "w1T")
            w2T = load_kxn(cpool, w2T_d, H, F16, "w2T")
            mb = cpool.tile([128, H], F16, tag="mb")
            nc.sync.dma_start(out=mb[:, :], in_=mb_d[:, :])
            ident = cpool.tile([128, 128], F16, tag="ident")
            masks.make_identity(nc, ident[:])

            pend_transp = None   # transposes of the previous t-tile
            pend_p45 = None      # phase 4+5 of the previous t-chunk

            pend_after_p45 = None
            p45_last_slot = False
            for b in range(B_LOC):
                inputT_h, contextT_v, context_v = state[b]
                htT = bpool.tile([128, KT * T], F16, tag="htT")
                htT_v = htT[:].rearrange("p (k t) -> p k t", k=KT)
                if b == 0:
                    emit_phase1(inputT_h, htT_v)
                else:
                    # second half's inputT slot frees only after the previous
                    # batch's phase-5(tc=1) — defer it behind pend_p45
                    emit_phase1(inputT_h, htT_v, chunks=[0])
                    pend_after_p45 = (
                        lambda ih=inputT_h, hv=htT_v:
                        emit_phase1(ih, hv, chunks=[1]))
                if pend_transp is not None:   # t7 of the previous batch
                    pend_transp[0]()
                    pend_transp[1]()
                    pend_transp = None

                for tc2 in range(TCH):
                    alignT = apool.tile([128, SBLK * 512], F16, tag="alignT")
                    alignT_v = alignT[:].rearrange("p (s t) -> p s t", s=SBLK)
                    for ts in range(4):
                        t_tile = tc2 * 4 + ts
                        # ---- phase 2: scores[t, s] for one t-tile ----
                        scores = wpool.tile([128, S], F32, tag="scores")
                        maxp = spool.tile([128, SCH], F32, tag="maxp")
                        for sc in range(SCH):
                            acc = ps.tile([128, 512], F32, tag="ps")
                            for k in range(KT):
                                nc.tensor.matmul(
                                    acc[:, :],
                                    htT_v[:, k, t_tile * 128:(t_tile + 1) * 128],
                                    contextT_v[:, k, sc * 512:(sc + 1) * 512],
                                    start=(k == 0), stop=(k == KT - 1))
                            # negated chunk-max first (critical path), then copy
                            nc.vector.tensor_reduce(
                                maxp[:, sc:sc + 1], acc[:, :],
                                axis=mybir.AxisListType.X,
                                op=mybir.AluOpType.max, negate=True)
                            if sc % 2 == 0:
                                nc.scalar.copy(scores[:, sc * 512:(sc + 1) * 512],
                                               acc[:, :])
                            else:
                                nc.vector.tensor_copy(
                                    scores[:, sc * 512:(sc + 1) * 512], acc[:, :])
                            if sc == 2 and pend_transp is not None:
                                pend_transp[0]()
                            elif sc == 3 and pend_transp is not None:
                                pend_transp[1]()
                                pend_transp = None
                        if p45_last_slot and pend_after_p45 is not None:
                            # one slot later still: scores+transposes cover
                            # the inputT1(b+1) DMA that frees at p45's end
                            pend_after_p45()
                            pend_after_p45 = None
                        p45_last_slot = False
                        if pend_p45 is not None:
                            pend_p45()
                            pend_p45 = None
                            p45_last_slot = True
                        # ---- phase 3: softmax pieces ----
                        negmax = spool.tile([128, 1], F32, tag="negmax")
                        nc.vector.tensor_reduce(
                            negmax[:, :], maxp[:, :], axis=mybir.AxisListType.X,
                            op=mybir.AluOpType.min)
                        expv = wpool.tile([128, S], F16, tag="expv")
                        rowsump = spool.tile([128, SCH], F32, tag="rowsump")
                        for sc in range(SCH):
                            nc.scalar.activation(
                                expv[:, sc * 512:(sc + 1) * 512],
                                scores[:, sc * 512:(sc + 1) * 512], AF.Exp,
                                bias=negmax[:, 0:1], scale=1.0,
                                accum_out=rowsump[:, sc:sc + 1])
                        rowsum = spool.tile([128, 1], F32, tag="rowsum")
                        nc.vector.tensor_reduce(
                            rowsum[:, :], rowsump[:, :], axis=mybir.AxisListType.X,
                            op=mybir.AluOpType.add)
                        recip = spool.tile([128, 1], F32, tag=f"recip{t_tile % 8}")
                        nc.vector.reciprocal(recip[:, :], rowsum[:, :])

                        def _quad(q, expv=expv, ts=ts, alignT_v=alignT_v):
                            # 8 f16 PE transposes per PSUM bank
                            ptr = ps.tile([128, 1024], F16, tag="ps")
                            ptr_v = ptr[:].rearrange("p (j t) -> p j t", j=8)
                            for j in range(8):
                                sb = q * 8 + j
                                nc.tensor.matmul(
                                    ptr_v[:, j, :],
                                    expv[:, sb * 128:(sb + 1) * 128],
                                    ident[:, :], is_transpose=True,
                                    start=(j == 0), stop=(j == 7))
                            for hq in range(2):
                                nc.scalar.copy(
                                    alignT_v[:, q * 8 + hq * 4:
                                             q * 8 + (hq + 1) * 4,
                                             ts * 128:(ts + 1) * 128],
                                    ptr_v[:, hq * 4:(hq + 1) * 4, :128])

                        pend_transp = [lambda f=_quad: f(0), lambda f=_quad: f(1)]
                        state.setdefault("recips", {})[(b, t_tile)] = recip

                    def pend_p45(b=b, tc2=tc2, alignT_v=alignT_v,
                                 inputT_h=inputT_h, context_v=context_v):
                        # ---- phase 4: cT'[h, t-chunk] ----
                        cT = apool.tile([128, KT * 512], F16, tag="cT")
                        cT_v = cT[:].rearrange("p (k t) -> p k t", k=KT)
                        for h in range(KT):
                            acc = ps.tile([128, 512], F32, tag="ps")
                            for sb in range(SBLK):
                                nc.tensor.matmul(
                                    acc[:, :],
                                    context_v[:, sb, h * 128:(h + 1) * 128],
                                    alignT_v[:, sb, :],
                                    start=(sb == 0), stop=(sb == SBLK - 1))
                            if h % 2 == 0:
                                nc.scalar.copy(cT_v[:, h, :], acc[:, :])
                            else:
                                nc.vector.tensor_copy(cT_v[:, h, :], acc[:, :])
                        # ---- phase 5: mlp + epilogue ----
                        for ts in range(4):
                            t_tile = tc2 * 4 + ts
                            recip = state["recips"][(b, t_tile)]
                            for oc in range(OCH):
                                psA = ps.tile([128, 512], F32, tag="ps")
                                for k in range(KT):
                                    nc.tensor.matmul(
                                        psA[:, :],
                                        cT_v[:, k, ts * 128:(ts + 1) * 128],
                                        w1T[:, k, oc * 512:(oc + 1) * 512],
                                        start=(k == 0), stop=(k == KT - 1))
                                psB = ps.tile([128, 512], F32, tag="ps")
                                for k in range(KT):
                                    nc.tensor.matmul(
                                        psB[:, :],
                                        inputT_h[tc2][:, k, ts * 128:(ts + 1) * 128],
                                        w2T[:, k, oc * 512:(oc + 1) * 512],
                                        start=(k == 0), stop=(k == KT - 1))
                                # out = tanh(psA*recip + mb + psB); the
                                # kernel-final tile is split in halves so its
                                # serial chain pipelines into the drain
                                last = (b == B_LOC - 1 and tc2 == TCH - 1)
                                tail = last and ts == 3 and oc == OCH - 1
                                sbA = opool.tile([128, 512], F32, tag="sbA")
                                sbC = opool.tile([128, 512], F32, tag="sbC")
                                for h0, h1 in ([(0, 256), (256, 512)]
                                               if tail else [(0, 512)]):
                                    nc.vector.scalar_tensor_tensor(
                                        sbA[:, h0:h1], psA[:, h0:h1],
                                        recip[:, 0:1],
                                        mb[:, oc * 512 + h0:oc * 512 + h1],
                                        op0=mybir.AluOpType.mult,
                                        op1=mybir.AluOpType.add)
                                    nc.vector.tensor_add(
                                        sbC[:, h0:h1], sbA[:, h0:h1],
                                        psB[:, h0:h1])
                                    nc.scalar.activation(
                                        sbC[:, h0:h1], sbC[:, h0:h1], AF.Tanh)
                                    oeng = nc.sync if last else nc.gpsimd
                                    oeng.dma_start(
                                        out=out_d[b * T + t_tile * 128:
                                                  b * T + (t_tile + 1) * 128,
                                                  oc * 512 + h0:oc * 512 + h1],
                                        in_=sbC[:, h0:h1])

                if b + 1 < B_LOC:
                    # prefetch next batch, phase 1 fills this batch's tail
                    # FIFO order matters: inputT half 1 waits on this batch's
                    # phase-5(tc=1) and would head-of-line-block the context
                    # loads, so it is emitted last
                    nih0 = load_inputT_half(b + 1, 0)
                    nctxT, nctx = load_contexts(b + 1)
                    nih1 = load_inputT_half(b + 1, 1)
                    state[b + 1] = ([nih0, nih1], nctxT, nctx)

            # tail flush
            if pend_transp is not None:
                pend_transp[0]()
                pend_transp[1]()
                pend_transp = None
            if pend_p45 is not None:
                pend_p45()
    nc.compile()
    return nc


def _prep_inputs(input, context, affine_w, affine_b, mlp_w, mlp_b):
    """Host-side sharding + layout prep. Returns in_maps for 8 cores."""
    awT = np.ascontiguousarray(affine_w.T).astype(np.float16)
    ab = np.ascontiguousarray(affine_b.reshape(H, 1)).astype(np.float32)
    w1T = np.ascontiguousarray(mlp_w[:, :H].T).astype(np.float16)
    w2T = np.ascontiguousarray(mlp_w[:, H:].T).astype(np.float16)
    mb = np.ascontiguousarray(np.broadcast_to(mlp_b.reshape(1, H), (128, H))).astype(np.float16)
    in_maps = []
    for c in range(N_CORES):
        gbs = [B_LOC * c + i for i in range(B_LOC)]
        inputT = np.stack([input[g].T for g in gbs]).astype(np.float16)
        contextT = np.stack([context[g].T for g in gbs]).astype(np.float16)
        ctx16 = np.stack([context[g] for g in gbs]).astype(np.float16)
        in_maps.append({
            "inputT": np.ascontiguousarray(inputT),
            "contextT": np.ascontiguousarray(contextT),
            "context": np.ascontiguousarray(ctx16),
            "affine_wT": awT, "affine_b": ab,
            "w1T": w1T, "w2T": w2T, "mlp_b": mb,
        })
    return in_maps


def get_nc():
    global _nc_cache
    if _nc_cache is None:
        _nc_cache = build()
    return _nc_cache


def kernel(input, context, affine_w, affine_b, mlp_w, mlp_b):
    input = np.asarray(input, dtype=np.float32)
    context = np.asarray(context, dtype=np.float32)
    affine_w = np.asarray(affine_w, dtype=np.float32)
    affine_b = np.asarray(affine_b, dtype=np.float32)
    mlp_w = np.asarray(mlp_w, dtype=np.float32)
    mlp_b = np.asarray(mlp_b, dtype=np.float32)

    nc = get_nc()
    in_maps = _prep_inputs(input, context, affine_w, affine_b, mlp_w, mlp_b)
    res = run_bass_kernel_spmd(nc, in_maps, core_ids=list(range(N_CORES)))
    out = np.empty((B, T, H), dtype=np.float32)
    for c in range(N_CORES):
        o = res.results[c]["out"]
        for i in range(B_LOC):
            out[B_LOC * c + i] = o[i * T:(i + 1) * T, :]
    return out


if __name__ == "__main__":
    rng = np.random.default_rng(0)
    ins = {
        "input": rng.standard_normal((B, T, H), dtype=np.float32),
        "context": rng.standard_normal((B, S, H), dtype=np.float32),
        "affine_w": rng.standard_normal((H, H), dtype=np.float32) / np.sqrt(H),
        "affine_b": rng.standard_normal((H,), dtype=np.float32) * 0.01,
        "mlp_w": rng.standard_normal((H, 2 * H), dtype=np.float32) / np.sqrt(2 * H),
        "mlp_b": rng.standard_normal((H,), dtype=np.float32) * 0.01,
    }
    out = kernel(**ins)
    print("kernel ran, out shape", out.shape, "finite:", np.isfinite(out).all())


# revision 58
# speedup vs baseline: 1.0029x; 1.0029x over previous
"""TRN2 Bass kernel for nn_Attention (Luong 'general' global attention).

reference:
    h_t    = input @ affine_w.T + affine_b          [B,T,H]
    scores = h_t @ context.T                        [B,T,S]
    align  = softmax(scores, axis=S)
    c      = align @ context                        [B,T,H]
    out    = tanh(concat([c, input]) @ mlp_w.T + mlp_b)

B=16, T=1024, S=2048, H=1024. Data-parallel over batch: 2 batches/core
on 8 NeuronCores, no collectives. Compute dtype fp16 (TensorE full
rate, f32 PSUM accumulate); softmax row-stats in f32.

Per-core dataflow (per local batch):
  1. h_tT[o,t]  = affine_wT-matmuls(inputT) + bias       (psum->sbuf fp16)
  2. scores[t,s] = h_tT.T @ contextT   per t-tile of 128 (psum->sbuf f32)
  3. rowmax/exp/rowsum via DVE reduce + ACT exp(accum_out), recip via DVE
  4. alignT[s,t] via PE transpose;  cT'[h,t] = context.T-matmuls(alignT)
  5. out[t,o] = tanh(cT'-matmuls*recip + inputT/W2-matmuls + mlp_b)

The emission order software-pipelines the PE stream across t-tiles,
t-chunks AND batches: transposes of tile i run behind the score matmuls
of tile i+1; phase 4+5 of chunk c run behind the first score group of
chunk c+1; phase 1 of batch b+1 fills the tail of batch b.
"""
import sys

sys.path.insert(0, "/opt/trn_rl_repo")
import numpy as np  # noqa: E402
from concourse import bacc, bass, tile, masks  # noqa: E402
from concourse.bass_utils import run_bass_kernel_spmd  # noqa: E402

mybir = bass.mybir
F16 = mybir.dt.float16
F32 = mybir.dt.float32
AF = mybir.ActivationFunctionType

N_CORES = 8
B, T, S, H = 16, 1024, 2048, 1024
B_LOC = B // N_CORES          # 2 batches per core
KT = H // 128                 # 8 contraction tiles
TT = T // 128                 # 8 t-tiles per batch
TCH = T // 512                # 2 t-chunks per batch
OCH = H // 512                # 2 output chunks
SCH = S // 512                # 4 score chunks
SBLK = S // 128               # 16 s blocks

_nc_cache = None


def build():
    nc = bacc.Bacc("TRN2", target_bir_lowering=False, debug=False,
                   num_devices=N_CORES)
    inputT_d = nc.declare_dram_parameter("inputT", [B_LOC, H, T], F16, isOutput=False)
    contextT_d = nc.declare_dram_parameter("contextT", [B_LOC, H, S], F16, isOutput=False)
    context_d = nc.declare_dram_parameter("context", [B_LOC, S, H], F16, isOutput=False)
    awT_d = nc.declare_dram_parameter("affine_wT", [H, H], F16, isOutput=False)
    ab_d = nc.declare_dram_parameter("affine_b", [H, 1], F32, isOutput=False)
    w1T_d = nc.declare_dram_parameter("w1T", [H, H], F16, isOutput=False)
    w2T_d = nc.declare_dram_parameter("w2T", [H, H], F16, isOutput=False)
    mb_d = nc.declare_dram_parameter("mlp_b", [128, H], F16, isOutput=False)
    out_d = nc.declare_dram_parameter("out", [B_LOC * T, H], F32, isOutput=True)

    with tile.TileContext(nc) as tc:
        with tc.tile_pool(name="const", bufs=1) as cpool, \
             tc.tile_pool(name="big", bufs=1) as bpool, \
             tc.tile_pool(name="align", bufs=1) as apool, \
             tc.tile_pool(name="work", bufs=2) as wpool, \
             tc.tile_pool(name="out", bufs=2) as opool, \
             tc.tile_pool(name="small", bufs=4) as spool, \
             tc.tile_pool(name="ps", bufs=8, space="PSUM") as ps:

            # HWDGE DMAs are FIFO per issuing engine (two rings: sync, scalar)
            # -> few big DMAs, critical ones first, spread over both rings.
            def load_kxn(pool, dram, n, dt, tag, groups=8, eng=None):
                t_ = pool.tile([128, KT * n], dt, tag=tag)
                v = t_[:].rearrange("p (k n) -> p k n", k=KT)
                kg = KT // groups
                for g in range(groups):
                    (eng or nc.sync).dma_start(
                        out=v[:, g * kg:(g + 1) * kg, :],
                        in_=dram[:].rearrange("(k p) n -> p k n", p=128)
                        [:, g * kg:(g + 1) * kg, :])
                return v

            def load_inputT_half(b, hh):
                ih = bpool.tile([128, KT * 512], F16, tag=f"inputT{hh}")
                ihv = ih[:].rearrange("p (k t) -> p k t", k=KT)
                for k in range(KT):
                    nc.sync.dma_start(
                        out=ihv[:, k, :],
                        in_=inputT_d[b].rearrange("(k p) t -> p k t", p=128)
                        [:, k, hh * 512:(hh + 1) * 512])
                return ihv

            def load_contexts(b):
                contextT = bpool.tile([128, KT * S], F16, tag="contextT")
                cTv = contextT[:].rearrange("p (k s) -> p k s", k=KT)
                for k in range(KT):
                    nc.sync.dma_start(
                        out=cTv[:, k, :],
                        in_=contextT_d[b].rearrange("(k p) s -> p k s", p=128)[:, k, :])
                context = bpool.tile([128, SBLK * H], F16, tag="context")
                cv = context[:].rearrange("p (s h) -> p s h", s=SBLK)
                for sb in range(SBLK):
                    nc.sync.dma_start(
                        out=cv[:, sb, :],
                        in_=context_d[b].rearrange("(s p) h -> p s h", p=128)[:, sb, :])
                return cTv, cv

            def emit_phase1(inputT_h, htT_v, chunks=None):
                for tc2 in (range(TCH) if chunks is None else chunks):
                    for o in range(KT):
                        acc = ps.tile([128, 512], F32, tag="ps")
                        for k in range(KT):
                            nc.tensor.matmul(
                                acc[:, :],
                                awT[:, k, o * 128:(o + 1) * 128],
                                inputT_h[tc2][:, k, :],
                                start=(k == 0), stop=(k == KT - 1))
                        nc.vector.tensor_scalar_add(
                            htT_v[:, o, tc2 * 512:(tc2 + 1) * 512],
                            acc[:, :], ab[:, o:o + 1])

            # ---- HAM pre-warm: PE busy during initial loads so the
            # clock gate opens (K=8/8) before the first real matmul ----
            warm = cpool.tile([128, 512], F16, tag="warm")
            nc.vector.memset(warm[:, :], 0.5)
            wps = ps.tile([128, 512], F32, tag="ps")
            for i in range(10):
                nc.tensor.matmul(wps[:, :], warm[:, 0:128], warm[:, :],
                                 start=(i == 0), stop=(i == 9))

            # ---- batch-0 critical-path loads first ----
            # awT split across both HWDGE rings (scalar ring idle at t=0)
            state = {}
            awT_t = cpool.tile([128, KT * H], F16, tag="awT")
            awT = awT_t[:].rearrange("p (k n) -> p k n", k=KT)
            for g in range(2):
                nc.sync.dma_start(
                    out=awT[:, g * 4:(g + 1) * 4, :],
                    in_=awT_d[:].rearrange("(k p) n -> p k n", p=128)
                    [:, g * 4:(g + 1) * 4, :])
            ab = cpool.tile([128, KT], F32, tag="ab")
            nc.sync.dma_start(out=ab[:, :],
                              in_=ab_d[:].rearrange("(o p) one -> p (o one)", p=128))
            # order: inputT half 0 (phase 1 tc0), contextT (first scores),
            # inputT half 1, context, then phase-5 weights
            ih0 = bpool.tile([128, KT * 512], F16, tag="inputT0")
            ih0v = ih0[:].rearrange("p (k t) -> p k t", k=KT)
            for g in range(2):
                nc.scalar.dma_start(
                    out=ih0v[:, g * 4:(g + 1) * 4, :],
                    in_=inputT_d[0].rearrange("(k p) t -> p k t", p=128)
                    [:, g * 4:(g + 1) * 4, 0:512])
            contextT0 = bpool.tile([128, KT * S], F16, tag="contextT")
            cTv0 = contextT0[:].rearrange("p (k s) -> p k s", k=KT)
            for k in range(KT):
                nc.sync.dma_start(
                    out=cTv0[:, k, :],
                    in_=contextT_d[0].rearrange("(k p) s -> p k s", p=128)[:, k, :])
            ih1 = bpool.tile([128, KT * 512], F16, tag="inputT1")
            ih1v = ih1[:].rearrange("p (k t) -> p k t", k=KT)
            for k in range(KT):
                nc.scalar.dma_start(
                    out=ih1v[:, k, :],
                    in_=inputT_d[0].rearrange("(k p) t -> p k t", p=128)
                    [:, k, 512:1024])
            context0 = bpool.tile([128, SBLK * H], F16, tag="context")
            cv0 = context0[:].rearrange("p (s h) -> p s h", s=SBLK)
            for sb in range(SBLK):
                nc.sync.dma_start(
                    out=cv0[:, sb, :],
                    in_=context_d[0].rearrange("(s p) h -> p s h", p=128)[:, sb, :])
            state[0] = ([ih0v, ih1v], cTv0, cv0)
            # phase-5 constants after the phase-1/2-critical DMAs
            w1T = load_kxn(cpool, w1T_d, H, F16, "w1T")
            w2T = load_kxn(cpool, w2T_d, H, F16, "w2T")
            mb = cpool.tile([128, H], F16, tag="mb")
            nc.sync.dma_start(out=mb[:, :], in_=mb_d[:, :])
            ident = cpool.tile([128, 128], F16, tag="ident")
            masks.make_identity(nc, ident[:])

            pend_transp = None   # transposes of the previous t-tile
            pend_p45 = None      # phase 4+5 of the previous t-chunk

            pend_after_p45 = None
            p45_last_slot = False
            for b in range(B_LOC):
                inputT_h, contextT_v, context_v = state[b]
                htT = bpool.tile([128, KT * T], F16, tag="htT")
                htT_v = htT[:].rearrange("p (k t) -> p k t", k=KT)
                if b == 0:
                    emit_phase1(inputT_h, htT_v)
                else:
                    # second half's inputT slot frees only after the previous
                    # batch's phase-5(tc=1) — defer it behind pend_p45
                    emit_phase1(inputT_h, htT_v, chunks=[0])
                    pend_after_p45 = (
                        lambda ih=inputT_h, hv=htT_v:
                        emit_phase1(ih, hv, chunks=[1]))
                if pend_transp is not None:   # t7 of the previous batch
                    pend_transp[0]()
                    pend_transp[1]()
                    pend_transp = None

                for tc2 in range(TCH):
                    alignT = apool.tile([128, SBLK * 512], F16, tag="alignT")
                    alignT_v = alignT[:].rearrange("p (s t) -> p s t", s=SBLK)
                    for ts in range(4):
                        t_tile = tc2 * 4 + ts
                        # ---- phase 2: scores[t, s] for one t-tile ----
                        scores = wpool.tile([128, S], F32, tag="scores")
                        maxp = spool.tile([128, SCH], F32, tag="maxp")
                        for sc in range(SCH):
                            acc = ps.tile([128, 512], F32, tag="ps")
                            for k in range(KT):
                                nc.tensor.matmul(
                                    acc[:, :],
                                    htT_v[:, k, t_tile * 128:(t_tile + 1) * 128],
                                    contextT_v[:, k, sc * 512:(sc + 1) * 512],
                                    start=(k == 0), stop=(k == KT - 1))
                            # negated chunk-max first (critical path), then copy
                            nc.vector.tensor_reduce(
                                maxp[:, sc:sc + 1], acc[:, :],
                                axis=mybir.AxisListType.X,
                                op=mybir.AluOpType.max, negate=True)
                            if sc % 2 == 0:
                                nc.scalar.copy(scores[:, sc * 512:(sc + 1) * 512],
                                               acc[:, :])
                            else:
                                nc.vector.tensor_copy(
                                    scores[:, sc * 512:(sc + 1) * 512], acc[:, :])
                            if sc == 2 and pend_transp is not None:
                                pend_transp[0]()
                            elif sc == 3 and pend_transp is not None:
                                pend_transp[1]()
                                pend_transp = None
                        if p45_last_slot and pend_after_p45 is not None:
                            # one slot later still: scores+transposes cover
                            # the inputT1(b+1) DMA that frees at p45's end
                            pend_after_p45()
                            pend_after_p45 = None
                        p45_last_slot = False
                        if pend_p45 is not None:
                            pend_p45()
                            pend_p45 = None
                            p45_last_slot = True
                        # ---- phase 3: softmax pieces ----
                        negmax = spool.tile([128, 1], F32, tag="negmax")
                        nc.vector.tensor_reduce(
                            negmax[:, :], maxp[:, :], axis=mybir.AxisListType.X,
                            op=mybir.AluOpType.min)
                        expv = wpool.tile([128, S], F16, tag="expv")
                        rowsump = spool.tile([128, SCH], F32, tag="rowsump")
                        for sc in range(SCH):
                            nc.scalar.activation(
                                expv[:, sc * 512:(sc + 1) * 512],
                                scores[:, sc * 512:(sc + 1) * 512], AF.Exp,
                                bias=negmax[:, 0:1], scale=1.0,
                                accum_out=rowsump[:, sc:sc + 1])
                        rowsum = spool.tile([128, 1], F32, tag="rowsum")
                        nc.vector.tensor_reduce(
                            rowsum[:, :], rowsump[:, :], axis=mybir.AxisListType.X,
                            op=mybir.AluOpType.add)
                        recip = spool.tile([128, 1], F32, tag=f"recip{t_tile % 8}")
                        nc.vector.reciprocal(recip[:, :], rowsum[:, :])

                        def _quad(q, expv=expv, ts=ts, alignT_v=alignT_v):
                            # 8 f16 PE transposes per PSUM bank
                            ptr = ps.tile([128, 1024], F16, tag="ps")
                            ptr_v = ptr[:].rearrange("p (j t) -> p j t", j=8)
                            for j in range(8):
                                sb = q * 8 + j
                                nc.tensor.matmul(
                                    ptr_v[:, j, :],
                                    expv[:, sb * 128:(sb + 1) * 128],
                                    ident[:, :], is_transpose=True,
                                    start=(j == 0), stop=(j == 7))
                            for hq in range(2):
                                nc.scalar.copy(
                                    alignT_v[:, q * 8 + hq * 4:
                                             q * 8 + (hq + 1) * 4,
                                             ts * 128:(ts + 1) * 128],
                                    ptr_v[:, hq * 4:(hq + 1) * 4, :128])

                        pend_transp = [lambda f=_quad: f(0), lambda f=_quad: f(1)]
                        state.setdefault("recips", {})[(b, t_tile)] = recip

                    def pend_p45(b=b, tc2=tc2, alignT_v=alignT_v,
                                 inputT_h=inputT_h, context_v=context_v):
                        # ---- phase 4: cT'[h, t-chunk] ----
                        cT = apool.tile([128, KT * 512], F16, tag="cT")
                        cT_v = cT[:].rearrange("p (k t) -> p k t", k=KT)
                        for h in range(KT):
                            acc = ps.tile([128, 512], F32, tag="ps")
                            for sb in range(SBLK):
                                nc.tensor.matmul(
                                    acc[:, :],
                                    context_v[:, sb, h * 128:(h + 1) * 128],
                                    alignT_v[:, sb, :],
                                    start=(sb == 0), stop=(sb == SBLK - 1))
                            if h % 2 == 0:
                                nc.scalar.copy(cT_v[:, h, :], acc[:, :])
                            else:
                                nc.vector.tensor_copy(cT_v[:, h, :], acc[:, :])
                        # ---- phase 5: mlp + epilogue ----
                        for ts in range(4):
                            t_tile = tc2 * 4 + ts
                            recip = state["recips"][(b, t_tile)]
                            for oc in range(OCH):
                                psA = ps.tile([128, 512], F32, tag="ps")
                                for k in range(KT):
                                    nc.tensor.matmul(
                                        psA[:, :],
                                        cT_v[:, k, ts * 128:(ts + 1) * 128],
                                        w1T[:, k, oc * 512:(oc + 1) * 512],
                                        start=(k == 0), stop=(k == KT - 1))
                                psB = ps.tile([128, 512], F32, tag="ps")
                                for k in range(KT):
                                    nc.tensor.matmul(
                                        psB[:, :],
                                        inputT_h[tc2][:, k, ts * 128:(ts + 1) * 128],
                                        w2T[:, k, oc * 512:(oc + 1) * 512],
                                        start=(k == 0), stop=(k == KT - 1))
                                # out = tanh(psA*recip + mb + psB)
                                sbA = opool.tile([128, 512], F32, tag="sbA")
                                nc.vector.scalar_tensor_tensor(
                                    sbA[:, :], psA[:, :], recip[:, 0:1],
                                    mb[:, oc * 512:(oc + 1) * 512],
                                    op0=mybir.AluOpType.mult,
                                    op1=mybir.AluOpType.add)
                                sbC = opool.tile([128, 512], F32, tag="sbC")
                                nc.vector.tensor_add(sbC[:, :], sbA[:, :], psB[:, :])
                                nc.scalar.activation(sbC[:, :], sbC[:, :], AF.Tanh)
                                oeng = (nc.sync if (b == B_LOC - 1 and tc2 == TCH - 1)
                                        else nc.gpsimd)
                                oeng.dma_start(
                                    out=out_d[b * T + t_tile * 128:
                                              b * T + (t_tile + 1) * 128,
                                              oc * 512:(oc + 1) * 512],
                                    in_=sbC[:, :])

                if b + 1 < B_LOC:
                    # prefetch next batch, phase 1 fills this batch's tail
                    # FIFO order matters: inputT half 1 waits on this batch's
                    # phase-5(tc=1) and would head-of-line-block the context
                    # loads, so it is emitted last
                    nih0 = load_inputT_half(b + 1, 0)
                    nctxT, nctx = load_contexts(b + 1)
                    nih1 = load_inputT_half(b + 1, 1)
                    state[b + 1] = ([nih0, nih1], nctxT, nctx)

            # tail flush
            if pend_transp is not None:
                pend_transp[0]()
                pend_transp[1]()
                pend_transp = None
            if pend_p45 is not None:
                pend_p45()
    nc.compile()
    return nc


def _prep_inputs(input, context, affine_w, affine_b, mlp_w, mlp_b):
    """Host-side sharding + layout prep. Returns in_maps for 8 cores."""
    awT = np.ascontiguousarray(affine_w.T).astype(np.float16)
    ab = np.ascontiguousarray(affine_b.reshape(H, 1)).astype(np.float32)
    w1T = np.ascontiguousarray(mlp_w[:, :H].T).astype(np.float16)
    w2T = np.ascontiguousarray(mlp_w[:, H:].T).astype(np.float16)
    mb = np.ascontiguousarray(np.broadcast_to(mlp_b.reshape(1, H), (128, H))).astype(np.float16)
    in_maps = []
    for c in range(N_CORES):
        gbs = [B_LOC * c + i for i in range(B_LOC)]
        inputT = np.stack([input[g].T for g in gbs]).astype(np.float16)
        contextT = np.stack([context[g].T for g in gbs]).astype(np.float16)
        ctx16 = np.stack([context[g] for g in gbs]).astype(np.float16)
        in_maps.append({
            "inputT": np.ascontiguousarray(inputT),
            "contextT": np.ascontiguousarray(contextT),
            "context": np.ascontiguousarray(ctx16),
            "affine_wT": awT, "affine_b": ab,
            "w1T": w1T, "w2T": w2T, "mlp_b": mb,
        })
    return in_maps


def get_nc():
    global _nc_cache
    if _nc_cache is None:
        _nc_cache = build()
    return _nc_cache


def kernel(input, context, affine_w, affine_b, mlp_w, mlp_b):
    input = np.asarray(input, dtype=np.float32)
    context = np.asarray(context, dtype=np.float32)
    affine_w = np.asarray(affine_w, dtype=np.float32)
    affine_b = np.asarray(affine_b, dtype=np.float32)
    mlp_w = np.asarray(mlp_w, dtype=np.float32)
    mlp_b = np.asarray(mlp_b, dtype=np.float32)

    nc = get_nc()
    in_maps = _prep_inputs(input, context, affine_w, affine_b, mlp_w, mlp_b)
    res = run_bass_kernel_spmd(nc, in_maps, core_ids=list(range(N_CORES)))
    out = np.empty((B, T, H), dtype=np.float32)
    for c in range(N_CORES):
        o = res.results[c]["out"]
        for i in range(B_LOC):
            out[B_LOC * c + i] = o[i * T:(i + 1) * T, :]
    return out


if __name__ == "__main__":
    rng = np.random.default_rng(0)
    ins = {
        "input": rng.standard_normal((B, T, H), dtype=np.float32),
        "context": rng.standard_normal((B, S, H), dtype=np.float32),
        "affine_w": rng.standard_normal((H, H), dtype=np.float32) / np.sqrt(H),
        "affine_b": rng.standard_normal((H,), dtype=np.float32) * 0.01,
        "mlp_w": rng.standard_normal((H, 2 * H), dtype=np.float32) / np.sqrt(2 * H),
        "mlp_b": rng.standard_normal((H,), dtype=np.float32) * 0.01,
    }
    out = kernel(**ins)
    print("kernel ran, out shape", out.shape, "finite:", np.isfinite(out).all())


# revision 60
# speedup vs baseline: 1.0034x; 1.0006x over previous
"""TRN2 Bass kernel for nn_Attention (Luong 'general' global attention).

reference:
    h_t    = input @ affine_w.T + affine_b          [B,T,H]
    scores = h_t @ context.T                        [B,T,S]
    align  = softmax(scores, axis=S)
    c      = align @ context                        [B,T,H]
    out    = tanh(concat([c, input]) @ mlp_w.T + mlp_b)

B=16, T=1024, S=2048, H=1024. Data-parallel over batch: 2 batches/core
on 8 NeuronCores, no collectives. Compute dtype fp16 (TensorE full
rate, f32 PSUM accumulate); softmax row-stats in f32.

Per-core dataflow (per local batch):
  1. h_tT[o,t]  = affine_wT-matmuls(inputT) + bias       (psum->sbuf fp16)
  2. scores[t,s] = h_tT.T @ contextT   per t-tile of 128 (psum->sbuf f32)
  3. rowmax/exp/rowsum via DVE reduce + ACT exp(accum_out), recip via DVE
  4. alignT[s,t] via PE transpose;  cT'[h,t] = context.T-matmuls(alignT)
  5. out[t,o] = tanh(cT'-matmuls*recip + inputT/W2-matmuls + mlp_b)

The emission order software-pipelines the PE stream across t-tiles,
t-chunks AND batches: transposes of tile i run behind the score matmuls
of tile i+1; phase 4+5 of chunk c run behind the first score group of
chunk c+1; phase 1 of batch b+1 fills the tail of batch b.
"""
import sys

sys.path.insert(0, "/opt/trn_rl_repo")
import numpy as np  # noqa: E402
from concourse import bacc, bass, tile, masks  # noqa: E402
from concourse.bass_utils import run_bass_kernel_spmd  # noqa: E402

mybir = bass.mybir
F16 = mybir.dt.float16
F32 = mybir.dt.float32
AF = mybir.ActivationFunctionType

N_CORES = 8
B, T, S, H = 16, 1024, 2048, 1024
B_LOC = B // N_CORES          # 2 batches per core
KT = H // 128                 # 8 contraction tiles
TT = T // 128                 # 8 t-tiles per batch
TCH = T // 512                # 2 t-chunks per batch
OCH = H // 512                # 2 output chunks
SCH = S // 512                # 4 score chunks
SBLK = S // 128               # 16 s blocks

_nc_cache = None


def build():
    nc = bacc.Bacc("TRN2", target_bir_lowering=False, debug=False,
                   num_devices=N_CORES)
    inputT_d = nc.declare_dram_parameter("inputT", [B_LOC, H, T], F16, isOutput=False)
    contextT_d = nc.declare_dram_parameter("contextT", [B_LOC, H, S], F16, isOutput=False)
    context_d = nc.declare_dram_parameter("context", [B_LOC, S, H], F16, isOutput=False)
    awT_d = nc.declare_dram_parameter("affine_wT", [H, H], F16, isOutput=False)
    ab_d = nc.declare_dram_parameter("affine_b", [H, 1], F32, isOutput=False)
    w1T_d = nc.declare_dram_parameter("w1T", [H, H], F16, isOutput=False)
    w2T_d = nc.declare_dram_parameter("w2T", [H, H], F16, isOutput=False)
    mb_d = nc.declare_dram_parameter("mlp_b", [128, H], F16, isOutput=False)
    out_d = nc.declare_dram_parameter("out", [B_LOC * T, H], F32, isOutput=True)

    with tile.TileContext(nc) as tc:
        with tc.tile_pool(name="const", bufs=1) as cpool, \
             tc.tile_pool(name="big", bufs=1) as bpool, \
             tc.tile_pool(name="align", bufs=1) as apool, \
             tc.tile_pool(name="work", bufs=2) as wpool, \
             tc.tile_pool(name="out", bufs=2) as opool, \
             tc.tile_pool(name="small", bufs=4) as spool, \
             tc.tile_pool(name="ps", bufs=8, space="PSUM") as ps:

            # HWDGE DMAs are FIFO per issuing engine (two rings: sync, scalar)
            # -> few big DMAs, critical ones first, spread over both rings.
            def load_kxn(pool, dram, n, dt, tag, groups=8, eng=None):
                t_ = pool.tile([128, KT * n], dt, tag=tag)
                v = t_[:].rearrange("p (k n) -> p k n", k=KT)
                kg = KT // groups
                for g in range(groups):
                    (eng or nc.sync).dma_start(
                        out=v[:, g * kg:(g + 1) * kg, :],
                        in_=dram[:].rearrange("(k p) n -> p k n", p=128)
                        [:, g * kg:(g + 1) * kg, :])
                return v

            def load_inputT_half(b, hh):
                ih = bpool.tile([128, KT * 512], F16, tag=f"inputT{hh}")
                ihv = ih[:].rearrange("p (k t) -> p k t", k=KT)
                for k in range(KT):
                    nc.sync.dma_start(
                        out=ihv[:, k, :],
                        in_=inputT_d[b].rearrange("(k p) t -> p k t", p=128)
                        [:, k, hh * 512:(hh + 1) * 512])
                return ihv

            def load_contexts(b):
                contextT = bpool.tile([128, KT * S], F16, tag="contextT")
                cTv = contextT[:].rearrange("p (k s) -> p k s", k=KT)
                for k in range(KT):
                    nc.sync.dma_start(
                        out=cTv[:, k, :],
                        in_=contextT_d[b].rearrange("(k p) s -> p k s", p=128)[:, k, :])
                context = bpool.tile([128, SBLK * H], F16, tag="context")
                cv = context[:].rearrange("p (s h) -> p s h", s=SBLK)
                for sb in range(SBLK):
                    nc.sync.dma_start(
                        out=cv[:, sb, :],
                        in_=context_d[b].rearrange("(s p) h -> p s h", p=128)[:, sb, :])
                return cTv, cv

            def emit_phase1(inputT_h, htT_v, chunks=None, ksplit=False):
                for tc2 in (range(TCH) if chunks is None else chunks):
                    if ksplit and tc2 == 0:
                        # startup: run all groups' k0-3 as soon as the first
                        # half of awT/inputT lands, k4-7 when the rest does
                        accs = [ps.tile([128, 512], F32, tag="ps",
                                        name=f"p1acc{o}")
                                for o in range(KT)]
                        for k in range(KT // 2):
                            for o in range(KT):
                                nc.tensor.matmul(
                                    accs[o][:, :],
                                    awT[:, k, o * 128:(o + 1) * 128],
                                    inputT_h[0][:, k, :],
                                    start=(k == 0), stop=False)
                        for k in range(KT // 2, KT):
                            for o in range(KT):
                                nc.tensor.matmul(
                                    accs[o][:, :],
                                    awT[:, k, o * 128:(o + 1) * 128],
                                    inputT_h[0][:, k, :],
                                    start=False, stop=(k == KT - 1))
                        for o in range(KT):
                            nc.vector.tensor_scalar_add(
                                htT_v[:, o, 0:512],
                                accs[o][:, :], ab[:, o:o + 1])
                        continue
                    for o in range(KT):
                        acc = ps.tile([128, 512], F32, tag="ps")
                        for k in range(KT):
                            nc.tensor.matmul(
                                acc[:, :],
                                awT[:, k, o * 128:(o + 1) * 128],
                                inputT_h[tc2][:, k, :],
                                start=(k == 0), stop=(k == KT - 1))
                        nc.vector.tensor_scalar_add(
                            htT_v[:, o, tc2 * 512:(tc2 + 1) * 512],
                            acc[:, :], ab[:, o:o + 1])

            # ---- HAM pre-warm: PE busy during initial loads so the
            # clock gate opens (K=8/8) before the first real matmul ----
            warm = cpool.tile([128, 512], F16, tag="warm")
            nc.vector.memset(warm[:, :], 0.5)
            wps = ps.tile([128, 512], F32, tag="ps")
            for i in range(10):
                nc.tensor.matmul(wps[:, :], warm[:, 0:128], warm[:, :],
                                 start=(i == 0), stop=(i == 9))

            # ---- batch-0 critical-path loads first ----
            # awT split across both HWDGE rings (scalar ring idle at t=0)
            state = {}
            awT_t = cpool.tile([128, KT * H], F16, tag="awT")
            awT = awT_t[:].rearrange("p (k n) -> p k n", k=KT)
            for g in range(2):
                nc.sync.dma_start(
                    out=awT[:, g * 4:(g + 1) * 4, :],
                    in_=awT_d[:].rearrange("(k p) n -> p k n", p=128)
                    [:, g * 4:(g + 1) * 4, :])
            ab = cpool.tile([128, KT], F32, tag="ab")
            nc.sync.dma_start(out=ab[:, :],
                              in_=ab_d[:].rearrange("(o p) one -> p (o one)", p=128))
            # order: inputT half 0 (phase 1 tc0), contextT (first scores),
            # inputT half 1, context, then phase-5 weights
            ih0 = bpool.tile([128, KT * 512], F16, tag="inputT0")
            ih0v = ih0[:].rearrange("p (k t) -> p k t", k=KT)
            for g in range(2):
                nc.scalar.dma_start(
                    out=ih0v[:, g * 4:(g + 1) * 4, :],
                    in_=inputT_d[0].rearrange("(k p) t -> p k t", p=128)
                    [:, g * 4:(g + 1) * 4, 0:512])
            contextT0 = bpool.tile([128, KT * S], F16, tag="contextT")
            cTv0 = contextT0[:].rearrange("p (k s) -> p k s", k=KT)
            for k in range(KT):
                nc.sync.dma_start(
                    out=cTv0[:, k, :],
                    in_=contextT_d[0].rearrange("(k p) s -> p k s", p=128)[:, k, :])
            ih1 = bpool.tile([128, KT * 512], F16, tag="inputT1")
            ih1v = ih1[:].rearrange("p (k t) -> p k t", k=KT)
            for k in range(KT):
                nc.scalar.dma_start(
                    out=ih1v[:, k, :],
                    in_=inputT_d[0].rearrange("(k p) t -> p k t", p=128)
                    [:, k, 512:1024])
            context0 = bpool.tile([128, SBLK * H], F16, tag="context")
            cv0 = context0[:].rearrange("p (s h) -> p s h", s=SBLK)
            for sb in range(SBLK):
                nc.sync.dma_start(
                    out=cv0[:, sb, :],
                    in_=context_d[0].rearrange("(s p) h -> p s h", p=128)[:, sb, :])
            state[0] = ([ih0v, ih1v], cTv0, cv0)
            # phase-5 constants after the phase-1/2-critical DMAs
            w1T = load_kxn(cpool, w1T_d, H, F16, "w1T")
            w2T = load_kxn(cpool, w2T_d, H, F16, "w2T")
            mb = cpool.tile([128, H], F16, tag="mb")
            nc.sync.dma_start(out=mb[:, :], in_=mb_d[:, :])
            ident = cpool.tile([128, 128], F16, tag="ident")
            masks.make_identity(nc, ident[:])

            pend_transp = None   # transposes of the previous t-tile
            pend_p45 = None      # phase 4+5 of the previous t-chunk

            pend_after_p45 = None
            p45_last_slot = False
            for b in range(B_LOC):
                inputT_h, contextT_v, context_v = state[b]
                htT = bpool.tile([128, KT * T], F16, tag="htT")
                htT_v = htT[:].rearrange("p (k t) -> p k t", k=KT)
                if b == 0:
                    emit_phase1(inputT_h, htT_v, ksplit=True)
                else:
                    # second half's inputT slot frees only after the previous
                    # batch's phase-5(tc=1) — defer it behind pend_p45
                    emit_phase1(inputT_h, htT_v, chunks=[0])
                    pend_after_p45 = (
                        lambda ih=inputT_h, hv=htT_v:
                        emit_phase1(ih, hv, chunks=[1]))
                if pend_transp is not None:   # t7 of the previous batch
                    pend_transp[0]()
                    pend_transp[1]()
                    pend_transp = None

                for tc2 in range(TCH):
                    alignT = apool.tile([128, SBLK * 512], F16, tag="alignT")
                    alignT_v = alignT[:].rearrange("p (s t) -> p s t", s=SBLK)
                    for ts in range(4):
                        t_tile = tc2 * 4 + ts
                        # ---- phase 2: scores[t, s] for one t-tile ----
                        scores = wpool.tile([128, S], F32, tag="scores")
                        maxp = spool.tile([128, SCH], F32, tag="maxp")
                        for sc in range(SCH):
                            acc = ps.tile([128, 512], F32, tag="ps")
                            for k in range(KT):
                                nc.tensor.matmul(
                                    acc[:, :],
                                    htT_v[:, k, t_tile * 128:(t_tile + 1) * 128],
                                    contextT_v[:, k, sc * 512:(sc + 1) * 512],
                                    start=(k == 0), stop=(k == KT - 1))
                            # negated chunk-max first (critical path), then copy
                            nc.vector.tensor_reduce(
                                maxp[:, sc:sc + 1], acc[:, :],
                                axis=mybir.AxisListType.X,
                                op=mybir.AluOpType.max, negate=True)
                            if sc % 2 == 0:
                                nc.scalar.copy(scores[:, sc * 512:(sc + 1) * 512],
                                               acc[:, :])
                            else:
                                nc.vector.tensor_copy(
                                    scores[:, sc * 512:(sc + 1) * 512], acc[:, :])
                            if sc == 2 and pend_transp is not None:
                                pend_transp[0]()
                            elif sc == 3 and pend_transp is not None:
                                pend_transp[1]()
                                pend_transp = None
                        if p45_last_slot and pend_after_p45 is not None:
                            # one slot later still: scores+transposes cover
                            # the inputT1(b+1) DMA that frees at p45's end
                            pend_after_p45()
                            pend_after_p45 = None
                        p45_last_slot = False
                        if pend_p45 is not None:
                            pend_p45()
                            pend_p45 = None
                            p45_last_slot = True
                        # ---- phase 3: softmax pieces ----
                        negmax = spool.tile([128, 1], F32, tag="negmax")
                        nc.vector.tensor_reduce(
                            negmax[:, :], maxp[:, :], axis=mybir.AxisListType.X,
                            op=mybir.AluOpType.min)
                        expv = wpool.tile([128, S], F16, tag="expv")
                        rowsump = spool.tile([128, SCH], F32, tag="rowsump")
                        for sc in range(SCH):
                            nc.scalar.activation(
                                expv[:, sc * 512:(sc + 1) * 512],
                                scores[:, sc * 512:(sc + 1) * 512], AF.Exp,
                                bias=negmax[:, 0:1], scale=1.0,
                                accum_out=rowsump[:, sc:sc + 1])
                        rowsum = spool.tile([128, 1], F32, tag="rowsum")
                        nc.vector.tensor_reduce(
                            rowsum[:, :], rowsump[:, :], axis=mybir.AxisListType.X,
                            op=mybir.AluOpType.add)
                        recip = spool.tile([128, 1], F32, tag=f"recip{t_tile % 8}")
                        nc.vector.reciprocal(recip[:, :], rowsum[:, :])

                        def _quad(q, expv=expv, ts=ts, alignT_v=alignT_v):
                            # 8 f16 PE transposes per PSUM bank
                            ptr = ps.tile([128, 1024], F16, tag="ps")
                            ptr_v = ptr[:].rearrange("p (j t) -> p j t", j=8)
                            for j in range(8):
                                sb = q * 8 + j
                                nc.tensor.matmul(
                                    ptr_v[:, j, :],
                                    expv[:, sb * 128:(sb + 1) * 128],
                                    ident[:, :], is_transpose=True,
                                    start=(j == 0), stop=(j == 7))
                            for hq in range(2):
                                nc.scalar.copy(
                                    alignT_v[:, q * 8 + hq * 4:
                                             q * 8 + (hq + 1) * 4,
                                             ts * 128:(ts + 1) * 128],
                                    ptr_v[:, hq * 4:(hq + 1) * 4, :128])

                        pend_transp = [lambda f=_quad: f(0), lambda f=_quad: f(1)]
                        state.setdefault("recips", {})[(b, t_tile)] = recip

                    def pend_p45(b=b, tc2=tc2, alignT_v=alignT_v,
                                 inputT_h=inputT_h, context_v=context_v):
                        # ---- phase 4: cT'[h, t-chunk] ----
                        cT = apool.tile([128, KT * 512], F16, tag="cT")
                        cT_v = cT[:].rearrange("p (k t) -> p k t", k=KT)
                        for h in range(KT):
                            acc = ps.tile([128, 512], F32, tag="ps")
                            for sb in range(SBLK):
                                nc.tensor.matmul(
                                    acc[:, :],
                                    context_v[:, sb, h * 128:(h + 1) * 128],
                                    alignT_v[:, sb, :],
                                    start=(sb == 0), stop=(sb == SBLK - 1))
                            if h % 2 == 0:
                                nc.scalar.copy(cT_v[:, h, :], acc[:, :])
                            else:
                                nc.vector.tensor_copy(cT_v[:, h, :], acc[:, :])
                        # ---- phase 5: mlp + epilogue ----
                        for ts in range(4):
                            t_tile = tc2 * 4 + ts
                            recip = state["recips"][(b, t_tile)]
                            for oc in range(OCH):
                                psA = ps.tile([128, 512], F32, tag="ps")
                                for k in range(KT):
                                    nc.tensor.matmul(
                                        psA[:, :],
                                        cT_v[:, k, ts * 128:(ts + 1) * 128],
                                        w1T[:, k, oc * 512:(oc + 1) * 512],
                                        start=(k == 0), stop=(k == KT - 1))
                                psB = ps.tile([128, 512], F32, tag="ps")
                                for k in range(KT):
                                    nc.tensor.matmul(
                                        psB[:, :],
                                        inputT_h[tc2][:, k, ts * 128:(ts + 1) * 128],
                                        w2T[:, k, oc * 512:(oc + 1) * 512],
                                        start=(k == 0), stop=(k == KT - 1))
                                # out = tanh(psA*recip + mb + psB)
                                sbA = opool.tile([128, 512], F32, tag="sbA")
                                nc.vector.scalar_tensor_tensor(
                                    sbA[:, :], psA[:, :], recip[:, 0:1],
                                    mb[:, oc * 512:(oc + 1) * 512],
                                    op0=mybir.AluOpType.mult,
                                    op1=mybir.AluOpType.add)
                                sbC = opool.tile([128, 512], F32, tag="sbC")
                                nc.vector.tensor_add(sbC[:, :], sbA[:, :], psB[:, :])
                                nc.scalar.activation(sbC[:, :], sbC[:, :], AF.Tanh)
                                oeng = (nc.sync if (b == B_LOC - 1 and tc2 == TCH - 1)
                                        else nc.gpsimd)
                                oeng.dma_start(
                                    out=out_d[b * T + t_tile * 128:
                                              b * T + (t_tile + 1) * 128,
                                              oc * 512:(oc + 1) * 512],
                                    in_=sbC[:, :])

                if b + 1 < B_LOC:
                    # prefetch next batch, phase 1 fills this batch's tail
                    # FIFO order matters: inputT half 1 waits on this batch's
                    # phase-5(tc=1) and would head-of-line-block the context
                    # loads, so it is emitted last
                    nih0 = load_inputT_half(b + 1, 0)
                    nctxT, nctx = load_contexts(b + 1)
                    nih1 = load_inputT_half(b + 1, 1)
                    state[b + 1] = ([nih0, nih1], nctxT, nctx)

            # tail flush
            if pend_transp is not None:
                pend_transp[0]()
                pend_transp[1]()
                pend_transp = None
            if pend_p45 is not None:
                pend_p45()
    nc.compile()
    return nc


def _prep_inputs(input, context, affine_w, affine_b, mlp_w, mlp_b):
    """Host-side sharding + layout prep. Returns in_maps for 8 cores."""
    awT = np.ascontiguousarray(affine_w.T).astype(np.float16)
    ab = np.ascontiguousarray(affine_b.reshape(H, 1)).astype(np.float32)
    w1T = np.ascontiguousarray(mlp_w[:, :H].T).astype(np.float16)
    w2T = np.ascontiguousarray(mlp_w[:, H:].T).astype(np.float16)
    mb = np.ascontiguousarray(np.broadcast_to(mlp_b.reshape(1, H), (128, H))).astype(np.float16)
    in_maps = []
    for c in range(N_CORES):
        gbs = [B_LOC * c + i for i in range(B_LOC)]
        inputT = np.stack([input[g].T for g in gbs]).astype(np.float16)
        contextT = np.stack([context[g].T for g in gbs]).astype(np.float16)
        ctx16 = np.stack([context[g] for g in gbs]).astype(np.float16)
        in_maps.append({
            "inputT": np.ascontiguousarray(inputT),
            "contextT": np.ascontiguousarray(contextT),
            "context": np.ascontiguousarray(ctx16),
            "affine_wT": awT, "affine_b": ab,
            "w1T": w1T, "w2T": w2T, "mlp_b": mb,
        })
    return in_maps


def get_nc():
    global _nc_cache
    if _nc_cache is None:
        _nc_cache = build()
    return _nc_cache


def kernel(input, context, affine_w, affine_b, mlp_w, mlp_b):
    input = np.asarray(input, dtype=np.float32)
    context = np.asarray(context, dtype=np.float32)
    affine_w = np.asarray(affine_w, dtype=np.float32)
    affine_b = np.asarray(affine_b, dtype=np.float32)
    mlp_w = np.asarray(mlp_w, dtype=np.float32)
    mlp_b = np.asarray(mlp_b, dtype=np.float32)

    nc = get_nc()
    in_maps = _prep_inputs(input, context, affine_w, affine_b, mlp_w, mlp_b)
    res = run_bass_kernel_spmd(nc, in_maps, core_ids=list(range(N_CORES)))
    out = np.empty((B, T, H), dtype=np.float32)
    for c in range(N_CORES):
        o = res.results[c]["out"]
        for i in range(B_LOC):
            out[B_LOC * c + i] = o[i * T:(i + 1) * T, :]
    return out


if __name__ == "__main__":
    rng = np.random.default_rng(0)
    ins = {
        "input": rng.standard_normal((B, T, H), dtype=np.float32),
        "context": rng.standard_normal((B, S, H), dtype=np.float32),
        "affine_w": rng.standard_normal((H, H), dtype=np.float32) / np.sqrt(H),
        "affine_b": rng.standard_normal((H,), dtype=np.float32) * 0.01,
        "mlp_w": rng.standard_normal((H, 2 * H), dtype=np.float32) / np.sqrt(2 * H),
        "mlp_b": rng.standard_normal((H,), dtype=np.float32) * 0.01,
    }
    out = kernel(**ins)
    print("kernel ran, out shape", out.shape, "finite:", np.isfinite(out).all())
